# revision 24
# baseline (speedup 1.0000x reference)
"""Block-recurrent transformer wrapper kernel for TRN2.

Strategy: data-parallel over batch (4 cores, one batch element each).
All activations live feature-major: [feature partitions, token free-dim].
Matmuls run as float32r (full-rate fp32) with PSUM accumulation.
Rotary is applied with host-permuted (even/odd grouped) Q/K weight columns,
so on device it is q_rot = cos*q + sin*(P@q) with P a signed swap matmul.
Softmax over the partition axis uses an appended ones-column on token-major V
to produce denominators inside the AV matmul itself.
"""

import sys

sys.path.insert(0, "/opt/trn_rl_repo")

import os
import numpy as np

import concourse.bass as bass
import concourse.mybir as mybir
import concourse.tile as tile
from concourse import bacc
from concourse.bass import ts
from concourse.bass_utils import run_bass_kernel_spmd

F32 = mybir.dt.float32
F32R = mybir.dt.float32r
AF = mybir.ActivationFunctionType
ALU = mybir.AluOpType

D, H, DH, NL, FF, S, W, I, O = 512, 8, 64, 4, 2048, 512, 512, 64, 64
REC = 1
NB = 8
DC = D // 128  # 4 feature chunks
SCALE = 1.0 / np.sqrt(DH)

LAST_EXEC_NS = [None]
GELU_F = None  # set at build time


def _r(ap):
    return ap.bitcast(F32R)


class Emitter:
    def __init__(self, nc, tc, NQ, nb):
        self.nc = nc
        self.tc = tc
        self.NQ = NQ
        self.nb = nb
        self.pools = {}

    def t(self, tag, shape=None, bufs=None, pool=None):
        pool = pool or self.pools["act"]
        shape = list(shape) if shape else [128, self.NQ]
        return pool.tile(shape, F32, tag=tag, name=tag, bufs=bufs)

    def gemm_fm(self, w_strips, x_tiles, F_out, n_free=None):
        """fm GEMM -> list of psum APs [128, n]."""
        nc = self.nc
        n = self.NQ if n_free is None else n_free
        outs = []
        nk = len(x_tiles)
        for fo in range(F_out // 128):
            ps = self.pools["psum"].tile([128, n], F32, tag="ps", name="ps")
            for k in range(nk):
                nc.tensor.matmul(
                    ps[:, :],
                    _r(w_strips[k][:, ts(fo, 128)]),
                    _r(x_tiles[k][:, :n]),
                    start=(k == 0),
                    stop=(k == nk - 1),
                )
            outs.append(ps)
        return outs

    def copy_to_sbuf(self, psums, tag, n=None, bufs=None):
        nc = self.nc
        n = n or self.NQ
        res = []
        for ps in psums:
            t = self.t(tag, [128, n], bufs=bufs)
            nc.scalar.copy(_r(t[:, :]), ps[0:128, :n])
            res.append(t)
        return res

    def layernorm(self, x_tiles, gamma_col, n=None):
        nc = self.nc
        n = n or self.NQ
        psum = self.pools["psum"]
        sq = []
        for x in x_tiles:
            t = self.t("sq", [128, n], bufs=4)
            nc.scalar.activation(_r(t[:, :]), x[:, :n], AF.Square)
            sq.append(t)
        ones = self.ones128
        ps_s = psum.tile([1, n], F32, tag="ps", name="ps")
        ps_q = psum.tile([1, n], F32, tag="ps", name="ps")
        for k in range(DC):
            nc.tensor.matmul(ps_s[0:1, :], _r(ones[:, 0:1]), _r(x_tiles[k][:, :n]),
                             start=(k == 0), stop=(k == DC - 1))
        for k in range(DC):
            nc.tensor.matmul(ps_q[0:1, :], _r(ones[:, 0:1]), _r(sq[k][:, :n]),
                             start=(k == 0), stop=(k == DC - 1))
        st = self.pools["stat"]
        m = st.tile([1, n], F32, tag="st", name="st")
        nc.scalar.activation(m[0:1, :], ps_s[0:1, :n], AF.Copy, scale=1.0 / D)
        msq = st.tile([1, n], F32, tag="st", name="st")
        nc.vector.tensor_mul(msq[0:1, :], m[0:1, :], m[0:1, :])
        var = st.tile([1, n], F32, tag="st", name="st")
        nc.vector.scalar_tensor_tensor(
            var[0:1, :], ps_q[0:1, :n], 1.0 / D, msq[0:1, :],
            op0=ALU.mult, op1=ALU.subtract)
        sd = st.tile([1, n], F32, tag="st", name="st")
        nc.scalar.activation(sd[0:1, :], var[0:1, :], AF.Sqrt,
                             bias=self.eps[0:1, 0:1])
        rs = st.tile([1, n], F32, tag="st", name="st")
        nc.vector.reciprocal(rs[0:1, :], sd[0:1, :])
        mb = self.t("bc", [128, n], bufs=4)
        rb = self.t("bc", [128, n], bufs=4)
        nc.gpsimd.partition_broadcast(mb[:, :], m[0:1, :n])
        nc.gpsimd.partition_broadcast(rb[:, :], rs[0:1, :n])
        outs = []
        for k in range(DC):
            xm = self.t("ln", [128, n], bufs=5)
            nc.vector.tensor_sub(_r(xm[:, :]), x_tiles[k][:, :n], mb[:, :])
            nc.vector.tensor_mul(_r(xm[:, :]), xm[:, :], rb[:, :])
            nc.vector.tensor_scalar_mul(_r(xm[:, :]), xm[:, :], gamma_col[:, k:k + 1])
            outs.append(xm)
        return outs

    def rotary_inplace(self, tiles, cos_t, sin_t, n=None):
        """Rotate tiles in place: x = cos*x + sin*(P@x)."""
        nc = self.nc
        n = n or self.NQ
        for x in tiles:
            sw = self.pools["psum"].tile([128, n], F32, tag="ps", name="ps")
            nc.tensor.matmul(sw[:, :], _r(self.PT[:, :]), _r(x[:, :n]),
                             start=True, stop=True)
            tmp = self.t("rtmp", [128, n], bufs=2)
            nc.vector.tensor_mul(tmp[:, :], sw[:, :], sin_t[:, :n])
            nc.vector.tensor_mul(_r(x[:, :n]), x[:, :n], cos_t[:, :n])
            nc.vector.tensor_add(_r(x[:, :n]), x[:, :n], tmp[:, :])
        return tiles

    def rotary_copy(self, src_tiles, cos_t, sin_t, out_tag, n=None, bufs=4):
        nc = self.nc
        n = n or self.NQ
        outs = []
        for x in src_tiles:
            sw = self.pools["psum"].tile([128, n], F32, tag="ps", name="ps")
            nc.tensor.matmul(sw[:, :], _r(self.PT[:, :]), _r(x[:, :n]),
                             start=True, stop=True)
            a = self.t(out_tag, [128, n], bufs=bufs)
            nc.vector.tensor_mul(_r(a[:, :]), x[:, :n], cos_t[:, :n])
            tmp = self.t("rtmp", [128, n], bufs=2)
            nc.vector.tensor_mul(tmp[:, :], sw[:, :], sin_t[:, :n])
            nc.vector.tensor_add(_r(a[:, :]), a[:, :], tmp[:, :])
            outs.append(a)
        return outs

    def attention(self, q_tiles, key_chunks, val_chunks, n_q=None,
                  mask_for=None):
        """key_chunks: list of (tiles4, kc); val_chunks: matching v_aug tiles
        [128, 8*65]. Returns 4 merged fm out tiles [128, n_q] (tag 'ao')."""
        nc = self.nc
        n = n_q or self.NQ
        psum = self.pools["psum"]
        nch = len(key_chunks)
        out_tiles = [self.t("ao", [128, n], bufs=8) for _ in range(DC)]
        for h in range(H):
            th, off = h // 2, 64 * (h % 2)
            av = psum.tile([128, n], F32, tag="ps", name="ps")
            for ci, (ktiles, kc) in enumerate(key_chunks):
                ps = psum.tile([128, n], F32, tag="ps", name="ps")
                nc.tensor.matmul(
                    ps[:, :],
                    _r(ktiles[th][off:off + 64, ts(kc, 128)]),
                    _r(q_tiles[th][off:off + 64, :n]),
                    start=True, stop=True,
                    tile_position=(off, 0),
                    skip_group_check=True,
                )
                e = self.t("exp", [128, n], bufs=3)
                nc.scalar.activation(_r(e[:, :]), ps[:, :], AF.Exp, scale=SCALE)
                if mask_for is not None and mask_for[ci] is not None:
                    nc.vector.tensor_mul(_r(e[:, :]), e[:, :], mask_for[ci][:, :n])
                nc.tensor.matmul(
                    av[0:65, :],
                    _r(val_chunks[ci][:, h * 65:h * 65 + 65]),
                    _r(e[:, :]),
                    start=(ci == 0), stop=(ci == nch - 1),
                    skip_group_check=True,
                )
            den = self.pools["stat"].tile([1, n], F32, tag="st", name="st")
            nc.vector.reciprocal(den[0:1, :], av[64:65, :])
            dbc = self.t("dbc", [64, n], bufs=2)
            nc.gpsimd.partition_broadcast(dbc[:, :], den[0:1, :n])
            nc.vector.tensor_mul(_r(out_tiles[th][off:off + 64, :]),
                                 av[0:64, :], dbc[:, :])
        return out_tiles

    def make_vaug(self, x_tiles, w_strips, n_rows, tag="vaug", bufs=4):
        nc = self.nc
        outs = []
        for sc in range(n_rows // 128):
            ps = self.pools["psum"].tile([128, 512], F32, tag="ps", name="ps")
            for k in range(DC):
                nc.tensor.matmul(
                    ps[:, :],
                    _r(x_tiles[k][:, ts(sc, 128)]),
                    _r(w_strips[k][:, :]),
                    start=(k == 0), stop=(k == DC - 1),
                )
            v = self.t(tag, [128, 8 * 65], bufs=bufs)
            v3 = v.rearrange("p (h u) -> p h u", u=65)
            nc.vector.tensor_copy(
                _r(v3[:, :, 0:64]),
                ps.rearrange("p (h u) -> p h u", u=64)[:, :, :])
            nc.sync.dma_start(out=_r(v3[:, :, 64:65]),
                              in_=self.onescol_d[:, :, None])
            outs.append(v)
        return outs


def build_nc(NQ=512, nb=NB, n_cores=4):
    nc = bacc.Bacc("TRN2", target_bir_lowering=False, debug=False,
                   enable_asserts=False, num_devices=n_cores)

    dram_in = {}

    def din(name, shape, dt=F32):
        dram_in[name] = nc.dram_tensor(name, list(shape), dt, kind="ExternalInput")
        return dram_in[name]

    xt = din("xt", (nb, I, W), F32R)
    emb = din("emb", (I, D), F32R)
    din("gammas", (128, DC * 10))
    for l in range(NL):
        din(f"Wq{l}", (D, D), F32R)
        din(f"Wk{l}", (D, D), F32R)
        din(f"Wv{l}", (D, D), F32R)
        if l != REC:
            din(f"Wo{l}", (D, D), F32R)
        din(f"W1_{l}", (D, FF), F32R)
        din(f"W2_{l}", (FF, D), F32R)
    din("rWsk", (D, D), F32R)
    din("rWsv", (D, D), F32R)
    din("rWsq", (D, D), F32R)
    din("rWo2", (2 * D, D), F32R)
    din("rWso", (2 * D, D), F32R)
    din("rWz", (D, D), F32R)
    din("Wlogit", (D, O), F32R)
    din("decay", (128, 2 * DC))
    din("state0", (D, S), F32R)
    din("PT", (128, 128), F32R)
    din("coshi", (128, W))
    din("sinhi", (128, W))
    din("coslo", (128, W))
    din("sinlo", (128, W))
    din("maskc", (DC, 128, W))
    din("ones128", (128, 1), F32R)
    din("onescol", (128, 8), F32R)

    out = nc.dram_tensor("out", [nb, O, NQ], F32, kind="ExternalOutput")
    dbg_on = bool(os.environ.get("BASS_DBG"))
    if dbg_on:
        dbg_h0 = nc.dram_tensor("dbg_h0", [nb, DC, 128, NQ], F32,
                                kind="ExternalOutput")
        dbg_st = nc.dram_tensor("dbg_st", [nb, DC, 128, S], F32,
                                kind="ExternalOutput")
        dbg_km = nc.dram_tensor("dbg_km", [nb, DC, 128, W], F32,
                                kind="ExternalOutput")
        dbg_ao = nc.dram_tensor("dbg_ao", [nb, DC, 128, NQ], F32,
                                kind="ExternalOutput")

    with tile.TileContext(nc) as tc:
        em = Emitter(nc, tc, NQ, nb)
        with (
            tc.tile_pool(name="const", bufs=1) as constp,
            tc.tile_pool(name="wp", bufs=9) as wp,
            tc.tile_pool(name="wff", bufs=5) as wff,
            tc.tile_pool(name="act", bufs=4) as actp,
            tc.tile_pool(name="stat", bufs=6) as statp,
            tc.tile_pool(name="hst", bufs=1) as hstp,
            tc.tile_pool(name="psum", bufs=8, space="PSUM") as psump,
            tc.tile_pool(name="dram", bufs=1, space="DRAM") as dramp,
        ):
            em.pools = {"act": actp, "psum": psump, "stat": statp}

            eps_t = constp.tile([1, 1], F32, tag="eps", name="eps")
            nc.vector.memset(eps_t[0:1, 0:1], 1e-5)
            em.eps = eps_t

            def load_const(name, shape, rnd=False):
                t = constp.tile(list(shape), F32, tag=name, name=name)
                o = _r(t[:, :]) if rnd else t[:, :]
                nc.sync.dma_start(out=o, in_=dram_in[name].ap())
                return t

            em.PT = load_const("PT", (128, 128), rnd=True)
            em.onescol_d = dram_in["onescol"]
            em.ones128 = load_const("ones128", (128, 1), rnd=True)
            coshi = load_const("coshi", (128, W))
            sinhi = load_const("sinhi", (128, W))
            coslo = load_const("coslo", (128, W))
            sinlo = load_const("sinlo", (128, W))
            gammas = load_const("gammas", (128, DC * 10))
            decay = load_const("decay", (128, 2 * DC))
            masks = []
            for c in range(DC):
                mt = constp.tile([128, W], F32, tag=f"mask{c}", name=f"mask{c}")
                nc.sync.dma_start(out=mt[:, :], in_=dram_in["maskc"][c])
                masks.append(mt)
            embt = constp.tile([I, D], F32, tag="emb", name="emb")
            nc.sync.dma_start(out=_r(embt[:, :]), in_=emb.ap())

            state = []
            for k in range(DC):
                st_t = hstp.tile([128, S], F32, tag=f"state{k}", name=f"state{k}")
                nc.sync.dma_start(out=_r(st_t[:, :]),
                                  in_=dram_in["state0"][ts(k, 128), :])
                state.append(st_t)

            kmem_d = []
            vmem_d = []
            for l in range(NL):
                km = dramp.tile([D, W], F32R, tag=f"km{l}", name=f"km{l}")
                vm = dramp.tile([W, 8 * 65], F32R, tag=f"vm{l}", name=f"vm{l}")
                kmem_d.append(km)
                vmem_d.append(vm)

            def wstrips(name, F_out, pool=None, rows=D):
                pool = pool or wp
                strips = []
                tag = "wff" if F_out > 1024 else "w"
                for k in range(rows // 128):
                    t = pool.tile([128, F_out], F32, tag=tag, name=tag)
                    nc.sync.dma_start(out=_r(t[:, :]), in_=dram_in[name][ts(k, 128), :])
                    strips.append(t)
                return strips

            for b in range(nb):
                first = (b == 0)
                xsb = em.t("x", [I, W], bufs=2)
                nc.sync.dma_start(out=_r(xsb[:, :]), in_=xt[b])
                h = []
                for fo in range(DC):
                    ps = psump.tile([128, NQ], F32, tag="ps", name="ps")
                    nc.tensor.matmul(ps[:, :], _r(embt[:, ts(fo, 128)]),
                                     _r(xsb[:, :NQ]), start=True, stop=True)
                    ht = em.t("h", [128, NQ], bufs=4)
                    nc.scalar.copy(_r(ht[:, :]), ps[:, :])
                    h.append(ht)

                for l in range(NL):
                    if dbg_on and l == 1:
                        for k in range(DC):
                            nc.sync.dma_start(out=dbg_h0[b, k], in_=h[k][:, :])
                    is_rec = (l == REC)
                    g1 = gammas[:, l * DC:(l + 1) * DC]
                    g2 = gammas[:, (NL + l) * DC:(NL + l + 1) * DC]
                    z = em.layernorm(h, g1)
                    wq = wstrips(f"Wq{l}", D)
                    q_ps = em.gemm_fm(wq, z, D)
                    q_raw = em.copy_to_sbuf(q_ps, "qraw", bufs=4)
                    wk = wstrips(f"Wk{l}", D)
                    k_ps = em.gemm_fm(wk, z, D)
                    k_raw = em.copy_to_sbuf(k_ps, "kraw", bufs=4)
                    wv = wstrips(f"Wv{l}", D)
                    v_aug = em.make_vaug(z, wv, W, tag="vaug", bufs=4)

                    # load previous block's memories BEFORE overwriting them
                    if not first:
                        km_raw = []
                        for k in range(DC):
                            t = em.t("kmemr", [128, W], bufs=4)
                            nc.sync.dma_start(out=_r(t[:, :]),
                                              in_=kmem_d[l][ts(k, 128), :])
                            km_raw.append(t)
                        vm = []
                        for k in range(DC):
                            t = em.t("vmem", [128, 8 * 65], bufs=4)
                            nc.sync.dma_start(out=_r(t[:, :]),
                                              in_=vmem_d[l][ts(k, 128), :])
                            vm.append(t)
                    # store current RAW k / v_aug (before in-place rotary)
                    if b < nb - 1:
                        for k in range(DC):
                            nc.sync.dma_start(out=kmem_d[l][ts(k, 128), :],
                                              in_=_r(k_raw[k][:, :]))
                            nc.sync.dma_start(out=vmem_d[l][ts(k, 128), :],
                                              in_=_r(v_aug[k][:, :]))

                    if is_rec:
                        q_rot = em.rotary_copy(q_raw, coshi, sinhi, "gel")
                        k_rot = em.rotary_copy(k_raw, coshi, sinhi, "bc")
                    else:
                        q_rot = em.rotary_inplace(q_raw, coshi, sinhi)
                        k_rot = em.rotary_inplace(k_raw, coshi, sinhi)

                    if not first:
                        km_rot = em.rotary_inplace(km_raw, coslo, sinlo)
                        key_chunks = [(km_rot, kc) for kc in range(DC)] + \
                                     [(k_rot, kc) for kc in range(DC)]
                        val_chunks = vm + v_aug
                        mask_for = [None] * DC + masks
                    else:
                        key_chunks = [(k_rot, kc) for kc in range(DC)]
                        val_chunks = v_aug
                        mask_for = masks

                    ao = em.attention(q_rot, key_chunks, val_chunks,
                                      mask_for=mask_for)


                    if is_rec:
                        gs = gammas[:, 8 * DC:9 * DC]
                        sz = em.layernorm(state, gs, n=S)
                        wsk = wstrips("rWsk", D)
                        sk_ps = em.gemm_fm(wsk, sz, D, n_free=S)
                        sk = em.copy_to_sbuf(sk_ps, "gel", n=S, bufs=4)
                        wsv = wstrips("rWsv", D)
                        sv_aug = em.make_vaug(sz, wsv, S, tag="vmem", bufs=4)
                        s_keys = [(sk, kc) for kc in range(DC)]
                        cross = em.attention(q_raw, s_keys, sv_aug)
                        wo2 = wstrips("rWo2", D, rows=2 * D)
                        o_ps = em.gemm_fm(wo2, ao + cross, D)
                        wsq = wstrips("rWsq", D)
                        sq_ps = em.gemm_fm(wsq, sz, D, n_free=S)
                        sqt = em.copy_to_sbuf(sq_ps, "sq", n=S, bufs=4)
                        s_self = em.attention(sqt, s_keys, sv_aug, n_q=S)
                        c_keys = [(k_raw, kc) for kc in range(DC)]
                        s_cross = em.attention(sqt, c_keys, v_aug, n_q=S)
                        wso = wstrips("rWso", D, rows=2 * D)
                        so_ps = em.gemm_fm(wso, s_self + s_cross, D, n_free=S)
                        so = em.copy_to_sbuf(so_ps, "kmemr", n=S, bufs=4)
                        wz = wstrips("rWz", D)
                        z_ps = em.gemm_fm(wz, so, D, n_free=S)
                        for k in range(DC):
                            nc.vector.tensor_scalar_mul(
                                _r(state[k][:, :]), state[k][:, :], decay[:, k:k + 1])
                            tnew = em.t("stnew", [128, S], bufs=2)
                            nc.vector.tensor_scalar_mul(
                                tnew[:, :], z_ps[k][0:128, :S],
                                decay[:, DC + k:DC + k + 1])
                            nc.vector.tensor_add(_r(state[k][:, :]),
                                                 state[k][:, :], tnew[:, :])
                        if dbg_on:
                            for k in range(DC):
                                nc.sync.dma_start(out=dbg_st[b, k],
                                                  in_=state[k][:, :])
                    else:
                        wo = wstrips(f"Wo{l}", D)
                        o_ps = em.gemm_fm(wo, ao, D)

                    for k in range(DC):
                        nc.vector.tensor_add(_r(h[k][:, :]), h[k][:, :],
                                             o_ps[k][0:128, :NQ])

                    f = em.layernorm(h, g2)
                    h2_ps = [psump.tile([128, NQ], F32, tag="ps", name="ps")
                             for _ in range(DC)]
                    for half in range(2):
                        w1 = []
                        for k in range(DC):
                            t = wff.tile([128, FF // 2], F32, tag="wff",
                                         name="wff")
                            nc.sync.dma_start(
                                out=_r(t[:, :]),
                                in_=dram_in[f"W1_{l}"][ts(k, 128),
                                                       half * (FF // 2):
                                                       (half + 1) * (FF // 2)])
                            w1.append(t)
                        for kk in range(FF // 256):
                            kc = half * (FF // 256) + kk
                            psf = psump.tile([128, NQ], F32, tag="ps", name="ps")
                            for k in range(DC):
                                nc.tensor.matmul(
                                    psf[:, :], _r(w1[k][:, ts(kk, 128)]),
                                    _r(f[k][:, :NQ]),
                                    start=(k == 0), stop=(k == DC - 1),
                                    skip_group_check=True)
                            gel = em.t("gel", [128, NQ], bufs=4)
                            gf_ = (AF.Tanh if os.environ.get("BASS_SIM_GELU")
                                   else AF.Gelu_apprx_tanh)
                            nc.scalar.activation(_r(gel[:, :]), psf[:, :], gf_)
                            w2s = wp.tile([128, D], F32, tag="w", name="w")
                            nc.sync.dma_start(
                                out=_r(w2s[:, :]),
                                in_=dram_in[f"W2_{l}"][ts(kc, 128), :])
                            for fo in range(DC):
                                nc.tensor.matmul(
                                    h2_ps[fo][:, :], _r(w2s[:, ts(fo, 128)]),
                                    _r(gel[:, :NQ]),
                                    start=(kc == 0),
                                    stop=(kc == FF // 128 - 1),
                                    skip_group_check=True)
                    for k in range(DC):
                        nc.vector.tensor_add(_r(h[k][:, :]), h[k][:, :],
                                             h2_ps[k][0:128, :NQ])

                gf = gammas[:, 9 * DC:10 * DC]
                hf = em.layernorm(h, gf)
                wlt = wstrips("Wlogit", O)
                lg = psump.tile([O, NQ], F32, tag="ps", name="ps")
                for k in range(DC):
                    nc.tensor.matmul(lg[0:O, :], _r(wlt[k][:, :O]),
                                     _r(hf[k][:, :NQ]),
                                     start=(k == 0), stop=(k == DC - 1))
                osb = em.t("osb", [O, NQ], bufs=2)
                nc.scalar.copy(osb[:, :], lg[0:O, :])
                nc.sync.dma_start(out=out[b], in_=osb[:, :])

    nc.finalize()
    return nc


# ---------------- host side ----------------

def _perm64():
    return np.concatenate([np.arange(0, 64, 2), np.arange(1, 64, 2)])


def _permD():
    p64 = _perm64()
    return np.concatenate([h * 64 + p64 for h in range(H)])


def _rotary_tables():
    inv_freq = 1.0 / (10000.0 ** (np.arange(0, DH, 2) / DH))
    pos = np.arange(2 * W)[:, None] * inv_freq[None, :]
    cos, sin = np.cos(pos), np.sin(pos)

    def tab(mm):
        blk = mm.T.astype(np.float32)
        return np.tile(blk, (4, 1))

    c, s = tab(cos), tab(sin)
    return (np.ascontiguousarray(c[:, W:]), np.ascontiguousarray(s[:, W:]),
            np.ascontiguousarray(c[:, :W]), np.ascontiguousarray(s[:, :W]))


def _pt_matrix():
    P = np.zeros((128, 128), np.float32)
    for g in range(2):
        o = g * 64
        for i in range(32):
            P[o + i, o + 32 + i] = -1.0
            P[o + 32 + i, o + i] = 1.0
    return np.ascontiguousarray(P.T)


def _masks():
    m = np.zeros((DC, 128, W), np.float32)
    for c in range(DC):
        for r in range(128):
            m[c, r, c * 128 + r:] = 1.0
    return m


def _gamma_col(g):
    return np.asarray(g, np.float32).reshape(DC, 128).T.copy()


def make_in_map(x_b, params, nb=NB):
    p = params
    perm = _permD()
    coshi, sinhi, coslo, sinlo = _rotary_tables()
    gam = np.zeros((128, DC * 10), np.float32)
    for l in range(NL):
        gam[:, l * DC:(l + 1) * DC] = _gamma_col(p["layers"][l]["ln1"])
        gam[:, (NL + l) * DC:(NL + l + 1) * DC] = _gamma_col(p["layers"][l]["ln2"])
    gam[:, 8 * DC:9 * DC] = _gamma_col(p["rec"]["ln_state"])
    gam[:, 9 * DC:10 * DC] = _gamma_col(p["ln_f"])

    dec = 1.0 / (1.0 + np.exp(-np.asarray(p["rec"]["ema_beta"], np.float64)))
    dec = dec.astype(np.float32)
    decay = np.zeros((128, 2 * DC), np.float32)
    decay[:, :DC] = _gamma_col(dec)
    decay[:, DC:] = _gamma_col(1.0 - dec)

    m = {
        "xt": np.ascontiguousarray(
            np.asarray(x_b, np.float32).reshape(nb, W, I).transpose(0, 2, 1)),
        "emb": np.asarray(p["emb"], np.float32).copy(),
        "gammas": gam,
        "decay": decay,
        "state0": np.ascontiguousarray(
            np.asarray(p["rec"]["init_state"], np.float32).T),
        "PT": _pt_matrix(),
        "coshi": coshi, "sinhi": sinhi, "coslo": coslo, "sinlo": sinlo,
        "maskc": _masks(),
        "ones128": np.ones((128, 1), np.float32),
        "onescol": np.ones((128, 8), np.float32),
        "Wlogit": np.asarray(p["Wlogit"], np.float32).copy(),
    }
    for l in range(NL):
        lp = p["layers"][l]
        m[f"Wq{l}"] = np.asarray(lp["Wq"], np.float32)[:, perm].copy()
        wkv = np.asarray(lp["Wkv"], np.float32)
        m[f"Wk{l}"] = wkv[:, :D][:, perm].copy()
        m[f"Wv{l}"] = wkv[:, D:].copy()
        if l != REC:
            m[f"Wo{l}"] = np.asarray(lp["Wo"], np.float32).copy()
        m[f"W1_{l}"] = np.asarray(lp["W1"], np.float32).copy()
        m[f"W2_{l}"] = np.asarray(lp["W2"], np.float32).copy()
    r = p["rec"]
    wskv = np.asarray(r["Wskv"], np.float32)
    m["rWsk"] = wskv[:, :D][:, perm].copy()
    m["rWsv"] = wskv[:, D:].copy()
    m["rWsq"] = np.asarray(r["Wsq"], np.float32)[:, perm].copy()
    m["rWo2"] = np.asarray(r["Wo2"], np.float32).copy()
    m["rWso"] = np.asarray(r["Wso"], np.float32).copy()
    m["rWz"] = np.asarray(r["Wz"], np.float32).copy()
    return m


def kernel(x_BLI, params):
    x = np.asarray(x_BLI, np.float32)
    B, L, _ = x.shape
    nb = L // W
    nc = build_nc(NQ=512, nb=nb, n_cores=B)
    in_maps = [make_in_map(x[b], params, nb) for b in range(B)]
    results, best_ns = run_timed(nc, in_maps, iters=4)
    LAST_EXEC_NS[0] = best_ns
    outs = []
    for b in range(B):
        o = results[b]["out"].reshape(nb, O, W)
        outs.append(o.transpose(0, 2, 1).reshape(L, O))
    return np.stack(outs, 0)


def run_timed(nc, in_maps, iters=5):
    """Compile once, then time repeated executions with device-resident
    inputs. Returns (results_list, best_wall_ns)."""
    import time
    import jax
    import jax.numpy as jnp
    from jax.sharding import Mesh, PartitionSpec
    from jax.experimental.shard_map import shard_map
    from concourse import bass2jax as b2j
    from concourse import mybir as _mb

    b2j.install_neuronx_cc_hook()
    n_cores = len(in_maps)
    partition_name = nc.partition_id_tensor.name if nc.partition_id_tensor else None
    in_names, out_names, out_avals, zero_outs = [], [], [], []
    for alloc in nc.m.functions[0].allocations:
        if not isinstance(alloc, _mb.MemoryLocationSet):
            continue
        name = alloc.memorylocations[0].name
        if alloc.kind == "ExternalInput":
            if name != partition_name:
                in_names.append(name)
        elif alloc.kind == "ExternalOutput":
            out_names.append(name)
            shape = tuple(alloc.tensor_shape)
            dtype = _mb.dt.np(alloc.dtype)
            out_avals.append(jax.core.ShapedArray(shape, dtype))
            zero_outs.append(np.zeros(shape, dtype))
    n_params = len(in_names)
    n_outs = len(out_avals)
    in_names_all = in_names + out_names
    if partition_name is not None:
        in_names_all = in_names_all + [partition_name]
    donate = tuple(range(n_params, n_params + n_outs))

    def _body(*args):
        operands = list(args)
        if partition_name is not None:
            operands.append(b2j.partition_id_tensor())
        outs = b2j._bass_exec_p.bind(
            *operands,
            out_avals=tuple(out_avals),
            in_names=tuple(in_names_all),
            out_names=tuple(out_names),
            lowering_input_output_aliases=(),
            sim_require_finite=True,
            sim_require_nnan=True,
            nc=nc,
        )
        return tuple(outs)

    devices = jax.devices()[:n_cores]
    mesh = Mesh(np.asarray(devices), ("core",))
    in_specs = (PartitionSpec("core"),) * (n_params + n_outs)
    out_specs = (PartitionSpec("core"),) * n_outs
    sharded = jax.jit(
        shard_map(_body, mesh=mesh, in_specs=in_specs, out_specs=out_specs,
                  check_rep=False),
        donate_argnums=donate, keep_unused=True)
    per_core = [[np.asarray(m[name]) for name in in_names] for m in in_maps]
    concat_in = [np.concatenate([per_core[c][i] for c in range(n_cores)], 0)
                 for i in range(n_params)]
    from jax.sharding import NamedSharding
    shard = NamedSharding(mesh, PartitionSpec("core"))
    dev_in = [jax.device_put(a, shard) for a in concat_in]

    def zeros():
        return [jax.device_put(
            np.zeros((n_cores * z.shape[0], *z.shape[1:]), z.dtype), shard)
            for z in zero_outs]

    out_arrs = sharded(*dev_in, *zeros())
    jax.block_until_ready(out_arrs)
    best = None
    for _ in range(iters):
        zs = zeros()
        jax.block_until_ready(zs)
        t0 = time.perf_counter()
        out_arrs2 = sharded(*dev_in, *zs)
        jax.block_until_ready(out_arrs2)
        dt = (time.perf_counter() - t0) * 1e9
        best = dt if best is None or dt < best else best
        out_arrs = out_arrs2
    results = [
        {name: np.asarray(out_arrs[i]).reshape(n_cores, *out_avals[i].shape)[c]
         for i, name in enumerate(out_names)}
        for c in range(n_cores)
    ]
    return results, best
